# revision 24
# baseline (speedup 1.0000x reference)
"""Masked-MLP (CorticalColumnMLP) Trainium2 kernel.

Math: out = gelu(x @ (w1*mask1).T, exact) @ (w2*mask2).T

Key structural fact: mask1 zeroes whole rows of w1 and mask2 zeroes whole
columns of w2 (2-of-4 structured sparsity). gelu(0) == 0, so only hidden
units j with mask1-row j kept AND mask2-col j kept contribute to the
output. Sharding therefore selects exactly those hidden units: the device
runs a dense MLP over the ~2065 surviving hidden units (padded to a
multiple of 128 with zeros, which is exact).

Distribution: pure data-parallel over tokens. Each of the 8 cores gets
1/8 of the 8192 tokens and the full (gathered) weight set; outputs are
disjoint token slices, concatenated on the host. No collectives.

Device layout (per core): everything is laid out so every DMA is
contiguous per partition and no transposes are needed on device:
  xt   [128, KD, Tc]   xt[p,k,t] = x[t, k*128+p]         (lhs, K=D on partitions)
  w1d  [JT, 128, KD, 128]  w1d[j,p,k,c] = W1g[j*128+c, k*128+p]
  w2d  [NT, 128, JT, 128]  w2d[n,p,j,c] = W2g.T[j*128+p, n*128+c]
  outt [NT, 128, Tc]   outt[n,p,t] = out[t, n*128+p]
Layer 1 computes hT[j,t] (hidden-major) so layer 2 can contract over
hidden without any transpose.
"""

import os

import numpy as np
import ml_dtypes

import concourse.bass as bass
import concourse.mybir as mybir
import concourse.tile as tile
from concourse import bacc
from concourse.bass import ts
from concourse.bass_utils import run_bass_kernel_spmd

P = 128
TS = 512  # matmul moving free dim / PSUM bank width (fp32)
N_CORES = 8

# "bf16" | "f16" | "f32r" | "f32" — device matmul dtype
MM_DTYPE = os.environ.get("BASS_MLP_DTYPE", "f16")

_DT = {
    "bf16": mybir.dt.bfloat16,
    "f16": mybir.dt.float16,
    "f32r": mybir.dt.float32r,
    "f32": mybir.dt.float32,
}
_NPDT = {
    "bf16": ml_dtypes.bfloat16,
    "f16": np.float16,
    "f32r": np.float32,
    "f32": np.float32,
}

# result of the last run_bass_kernel_spmd call (for test harness inspection)
LAST_RESULT = None

_NC_CACHE = {}


def _build_nc(D, Hg, Tc, mode, act="Gelu"):
    """Build + compile the per-core Bass program (dense MLP, hidden=Hg)."""
    act_fn = getattr(mybir.ActivationFunctionType, act)
    dt_in = _DT[mode]
    f32 = mybir.dt.float32
    KD = D // P
    JT = Hg // P
    NT = D // P
    nTS = Tc // TS
    assert D % P == 0 and Hg % P == 0 and Tc % TS == 0

    # phase-A wave width (j-tiles processed k-major together). Waves let the
    # first matmuls start as soon as the first x k-chunk lands instead of
    # waiting for all of x; wave*nTS PSUM groups are in flight at once.
    four_byte = mode in ("f32r", "f32")
    JW = 3 if four_byte else 4
    w1_bufs = JW + 1 if four_byte else 2 * JW
    w2_bufs = 2 if four_byte else 4
    o_bufs = 2 if four_byte else 4

    nc = bacc.Bacc("TRN2", target_bir_lowering=False, debug=False,
                   num_devices=N_CORES)
    xt = nc.dram_tensor("xt", [P, KD, Tc], dt_in, kind="ExternalInput")
    w1d = nc.dram_tensor("w1d", [JT, P, KD, P], dt_in, kind="ExternalInput")
    w2d = nc.dram_tensor("w2d", [NT, P, JT, P], dt_in, kind="ExternalInput")
    outt = nc.dram_tensor("outt", [NT, P, Tc], f32, kind="ExternalOutput")

    with tile.TileContext(nc) as tc:
        with (
            tc.tile_pool(name="xp", bufs=1) as xp,
            tc.tile_pool(name="w1p", bufs=w1_bufs) as w1p,
            tc.tile_pool(name="w2p", bufs=w2_bufs) as w2p,
            tc.tile_pool(name="hp", bufs=1) as hp,
            tc.tile_pool(name="op", bufs=o_bufs) as op,
            tc.tile_pool(name="pp", bufs=8, space="PSUM") as pp,
            tc.tile_pool(name="wup", bufs=1) as wup,
        ):
            # PE warm-up: trivial matmuls on scratch data keep the PE busy
            # from t~6.5us so the HAM clock-gate ramps (0.65->1.2->2.4 GHz)
            # while the first w1/x DMAs are in flight (~1.8us). Sized so the
            # warmup queue drains right as the first tile lands; more would
            # delay the first real matmul (tensor queue is FIFO). fp32 tile:
            # fp16/bf16 MEMSET is not a valid ISA instruction.
            n_warm = int(os.environ.get("BASS_MLP_WARMUP", "10"))
            wu = wup.tile([P, P], f32)
            nc.gpsimd.memset(wu, 0.0)
            wups = pp.tile([P, TS], f32, tag="ps", name="warm_ps")
            for _ in range(n_warm):
                nc.tensor.matmul(wups[:, :P], lhsT=wu, rhs=wu,
                                 start=True, stop=True)

            x_tile = xp.tile([P, KD, Tc], dt_in)
            hT = hp.tile([P, JT, Tc], dt_in)

            # Layer 1: hT[j_tile, t] = gelu(sum_k w1.T @ x), in waves of JW
            # j-tiles, k-major so matmuls chase the x k-chunk DMA stream.
            # DMA emission order (single HWDGE ring => FIFO landing):
            # first wave's w1 blocks, then the x chunks in k order.
            first = True
            for w0 in range(0, JT, JW):
                js = list(range(w0, min(w0 + JW, JT)))
                w1ts = {}
                for j in js:
                    w1ts[j] = w1p.tile([P, KD, P], dt_in, tag="w1", name=f"w1t{j}")
                if first:
                    # First wave: issue w1 in k-slabs interleaved across j so
                    # the k=0 blocks of ALL wave tiles land first (the k-major
                    # matmul order needs every j's k-slab almost immediately;
                    # whole-tile DMAs made matmul #4+ stall on w1t1..3).
                    QS = 4
                    for q in range(0, KD, QS):
                        for j in js:
                            nc.scalar.dma_start(w1ts[j][:, q:q + QS, :],
                                                w1d[j][:, q:q + QS, :])
                    for k in range(KD):
                        nc.sync.dma_start(x_tile[:, k, :], xt[:, k, :])
                    first = False
                else:
                    for j in js:
                        # scalar (2nd HWDGE ring): w1 lands in parallel with x
                        nc.scalar.dma_start(w1ts[j], w1d[j])
                pss = {}
                for j in js:
                    for t in range(nTS):
                        pss[j, t] = pp.tile([P, TS], f32, tag="ps",
                                            name=f"psA{j}_{t}")
                for k in range(KD):
                    for j in js:
                        for t in range(nTS):
                            nc.tensor.matmul(
                                pss[j, t], lhsT=w1ts[j][:, k, :],
                                rhs=x_tile[:, k, ts(t, TS)],
                                start=(k == 0), stop=(k == KD - 1),
                            )
                for j in js:
                    for t in range(nTS):
                        nc.scalar.activation(hT[:, j, ts(t, TS)], pss[j, t],
                                             act_fn)

            # Layer 2: outT[n_tile, t] = sum_j w2g.T @ hT
            # The very last chunk is split in half so the post-last-matmul
            # critical chain (psum copy + out DMA + completion) is halved:
            # the first half's copy/DMA overlaps the second half's matmuls.
            for n in range(NT):
                w2t = w2p.tile([P, JT, P], dt_in, tag="w2")
                nc.scalar.dma_start(w2t, w2d[n])
                for t in range(nTS):
                    last = (n == NT - 1) and (t == nTS - 1)
                    nsplit = 2 if last else 1
                    W = TS // nsplit
                    for h in range(nsplit):
                        ps = pp.tile([P, W], f32, tag="ps")
                        for j in range(JT):
                            nc.tensor.matmul(
                                ps, lhsT=w2t[:, j, :],
                                rhs=hT[:, j, t * TS + h * W:t * TS + (h + 1) * W],
                                start=(j == 0), stop=(j == JT - 1),
                            )
                        ot = op.tile([P, W], f32, tag="o")
                        nc.vector.tensor_copy(ot, ps)
                        nc.sync.dma_start(
                            outt[n, :, t * TS + h * W:t * TS + (h + 1) * W], ot)

    nc.compile()
    return nc


def _get_nc(D, Hg, Tc, mode):
    key = (D, Hg, Tc, mode)
    if key not in _NC_CACHE:
        _NC_CACHE[key] = _build_nc(D, Hg, Tc, mode)
    return _NC_CACHE[key]


def _build_nc_strassen(Tc):
    """Strassen layer-1 + standard layer-2, f16 only.

    L1: H[2304, 2048tok?] -- W[2304,2048] @ X[2048,Tc] via 2x2x2 Strassen:
    rows split 1152|1152 (9 tiles each), K split 1024|1024, cols Tc/2|Tc/2.
    7 products M1..M7 of shape [1152, Tc/2] each contract 8 k-tiles:
    504 matmuls instead of 544 (17-tile dense) for the same H.
    M2/M6 are computed first in an 8-wide j-wave (they only need the first
    half of x / cheap combos) and staged to SBUF f32; the other five run
    per-j with Vector folds into SBUF f32 quadrant accumulators, then gelu.
    Hidden-unit tile indices are unchanged (1152 = 9*128), so L2 is the
    usual dense contraction over tiles 0..16 (tile 17 is all padding).
    """
    D = 2048
    TS = 512
    KD = D // P          # 16 x k-chunks
    KH = KD // 2         # 8 k-tiles per Strassen half
    JH = 9               # j-tiles per row half (1152/128)
    NT = D // P
    JL2 = 17             # L2 contraction tiles (2065 real rows < 2176)
    nTS = Tc // TS
    assert Tc == 1024 and nTS == 2
    f32 = mybir.dt.float32
    f16 = mybir.dt.float16
    AL = mybir.AluOpType
    act_fn = mybir.ActivationFunctionType.Gelu

    nc = bacc.Bacc("TRN2", target_bir_lowering=False, debug=False,
                   num_devices=N_CORES)
    xt = nc.dram_tensor("xt", [P, KD, Tc], f16, kind="ExternalInput")
    w1s = nc.dram_tensor("w1s", [7, JH, P, KH, P], f16, kind="ExternalInput")
    w2s = nc.dram_tensor("w2s", [7, NT // 2, P, JH, P], f16,
                         kind="ExternalInput")
    outt = nc.dram_tensor("outt", [NT, P, Tc], f32, kind="ExternalOutput")

    with tile.TileContext(nc) as tc:
        with (
            tc.tile_pool(name="xp", bufs=1) as xp,
            tc.tile_pool(name="w1p", bufs=10) as w1p,
            tc.tile_pool(name="w2p", bufs=10) as w2p,
            tc.tile_pool(name="hcp", bufs=1) as hcp,
            tc.tile_pool(name="hp", bufs=1) as hp,
            tc.tile_pool(name="msp", bufs=1) as msp,
            tc.tile_pool(name="xcp", bufs=1) as xcp,
            tc.tile_pool(name="hap", bufs=4) as hap,
            tc.tile_pool(name="pp", bufs=8, space="PSUM") as pp,
            tc.tile_pool(name="wup", bufs=1) as wup,
        ):
            n_warm = int(os.environ.get("BASS_MLP_WARMUP", "10"))
            wu = wup.tile([P, P], f32)
            nc.gpsimd.memset(wu, 0.0)
            wups = pp.tile([P, TS], f32, tag="ps", name="warm_ps")
            for _ in range(n_warm):
                nc.tensor.matmul(wups[:, :P], lhsT=wu, rhs=wu,
                                 start=True, stop=True)

            x_tile = xp.tile([P, KD, Tc], f16)
            hT = hp.tile([P, 2 * JH - 1, Tc], f16)   # tile 17 is never used
            m2sb = msp.tile([P, JH - 1, TS], f16, tag="m2")
            # combos: XC1=X11+X22  XC3=X12-X22  XC4=X21-X11  XC6=X11+X12
            #         XC7=X21+X22
            xc1 = xcp.tile([P, KH, TS], f16, tag="xc1")
            xc3 = xcp.tile([P, KH, TS], f16, tag="xc3")
            xc4 = xcp.tile([P, KH, TS], f16, tag="xc4")
            xc6 = xcp.tile([P, KH, TS], f16, tag="xc6")
            xc7 = xcp.tile([P, KH, TS], f16, tag="xc7")
            # h-combos for the Strassen layer 2 (built during phase B):
            # HC1=HH11+HH22  HC3=HH12-HH22  HC4=HH21-HH11  HC6=HH11+HH12
            # HC7=HH21+HH22 ; kk=8 pieces reduce to raw hT[:,8,:] slices
            # (bottom tile 17 is all padding) or are skipped entirely.
            hc1 = hcp.tile([P, KH, TS], f16, tag="hc1")
            hc3 = hcp.tile([P, KH, TS], f16, tag="hc3")
            hc4 = hcp.tile([P, KH, TS], f16, tag="hc4")
            hc6 = hcp.tile([P, JH, TS], f16, tag="hc6")
            hc7 = hcp.tile([P, KH, TS], f16, tag="hc7")

            # phase A weights (M2 = C2 @ X11): k-slabs interleaved across j
            # so every j's k=0 slab lands early for the k-outer wave.
            w1A = [w1p.tile([P, KH, P], f16, tag="w1", name=f"w1A{j}")
                   for j in range(JH - 1)]
            QS = 4
            for q in range(0, KH, QS):
                for j in range(JH - 1):
                    nc.scalar.dma_start(w1A[j][:, q:q + QS, :],
                                        w1s[1, j][:, q:q + QS, :])
            # x half-chunks in need order: phase A consumes only the
            # t0-halves of chunks 0-7, so deliver those first (doubles the
            # effective delivery rate for the phase-A chase); combo inputs
            # and X22 follow with ample margin.
            for k in range(KH):
                nc.sync.dma_start(x_tile[:, k, 0:TS], xt[:, k, 0:TS])
            for k in range(KH):
                nc.sync.dma_start(x_tile[:, k, TS:2 * TS], xt[:, k, TS:2 * TS])
            for k in range(KH):
                nc.sync.dma_start(x_tile[:, 8 + k, 0:TS], xt[:, 8 + k, 0:TS])
            for k in range(KH):
                nc.sync.dma_start(x_tile[:, 8 + k, TS:2 * TS],
                                  xt[:, 8 + k, TS:2 * TS])

            # x-combos on the vector engine (it is idle until phase B;
            # gpsimd tensor ops turned out to be several-fold slower and
            # starved phase B). Emitted k-outer in arrival order of the
            # chunks each combo needs.
            for k in range(KH):
                nc.vector.tensor_tensor(xc6[:, k, :], x_tile[:, k, 0:TS],
                                        x_tile[:, k, TS:2 * TS], AL.add)
            for k in range(KH):
                nc.vector.tensor_tensor(xc4[:, k, :], x_tile[:, 8 + k, 0:TS],
                                        x_tile[:, k, 0:TS], AL.subtract)
                nc.vector.tensor_tensor(xc1[:, k, :], x_tile[:, k, 0:TS],
                                        x_tile[:, 8 + k, TS:2 * TS], AL.add)
                nc.vector.tensor_tensor(xc7[:, k, :], x_tile[:, 8 + k, 0:TS],
                                        x_tile[:, 8 + k, TS:2 * TS], AL.add)
                nc.vector.tensor_tensor(xc3[:, k, :], x_tile[:, k, TS:2 * TS],
                                        x_tile[:, 8 + k, TS:2 * TS],
                                        AL.subtract)

            # phase A: M2[j] = C2[j] @ X11 for j=0..7 (j=8 feeds only the
            # all-padding bottom quadrants -> skipped), 8-wide k-outer wave
            m2ps = [pp.tile([P, TS], f32, tag="ps", name=f"m2ps{j}")
                    for j in range(8)]
            for k in range(KH):
                for j in range(8):
                    nc.tensor.matmul(m2ps[j], lhsT=w1A[j][:, k, :],
                                     rhs=x_tile[:, k, 0:TS],
                                     start=(k == 0), stop=(k == KH - 1))
            # phase B: per j the remaining Mi + vector folds + gelu.
            # M6 first (its combo xc6 is ready earliest), M3 last.
            B_ORDER = [5, 3, 4, 0, 6, 2]  # M6 M4 M5 M1 M7 M3

            w1pre = {}
            for j in range(8):
                nc.scalar.copy(m2sb[:, j, :], m2ps[j])
            rhs_of = {
                5: lambda k: xc6[:, k, :],
                3: lambda k: xc4[:, k, :],
                4: lambda k: x_tile[:, 8 + k, TS:2 * TS],  # X22 raw
                0: lambda k: xc1[:, k, :],
                6: lambda k: xc7[:, k, :],
                2: lambda k: xc3[:, k, :],
            }
            for j in range(JH):
                bot = j < JH - 1   # j==8 bottom quadrants are all padding
                w1t = {}
                order = B_ORDER if bot else [3, 4, 0, 6, 2]
                for mi in order:
                    if (j, mi) in w1pre:
                        w1t[mi] = w1pre[j, mi]
                        continue
                    w1t[mi] = w1p.tile([P, KH, P], f16, tag="w1",
                                       name=f"w1B{mi}_{j}")
                    nc.scalar.dma_start(w1t[mi], w1s[mi, j])
                for mi in order:
                    p_ = pp.tile([P, TS], f32, tag="ps", name=f"mps{mi}_{j}")
                    for k in range(KH):
                        nc.tensor.matmul(p_, lhsT=w1t[mi][:, k, :],
                                         rhs=rhs_of[mi](k),
                                         start=(k == 0), stop=(k == KH - 1))
                    if mi == 5:    # M6 done (bot only)
                        h22 = hap.tile([P, TS], f32, tag="ha", name=f"h22_{j}")
                        nc.vector.tensor_copy(h22, p_)
                    elif mi == 3:  # M4 done
                        h11 = hap.tile([P, TS], f32, tag="ha", name=f"h11_{j}")
                        nc.vector.tensor_copy(h11, p_)
                        if bot:
                            h21 = hap.tile([P, TS], f32, tag="ha",
                                           name=f"h21_{j}")
                            nc.vector.tensor_tensor(h21, m2sb[:, j, :], p_,
                                                    AL.add)
                            nc.scalar.activation(hT[:, JH + j, 0:TS], h21,
                                                 act_fn)
                    elif mi == 4:  # M5 done
                        h12 = hap.tile([P, TS], f32, tag="ha", name=f"h12_{j}")
                        nc.vector.tensor_tensor(h11, h11, p_, AL.subtract)
                        nc.vector.tensor_copy(h12, p_)
                    elif mi == 0:  # M1 done
                        nc.vector.tensor_tensor(h11, h11, p_, AL.add)
                        if bot:
                            nc.vector.tensor_tensor(h22, h22, p_, AL.add)
                    elif mi == 6:  # M7 done -> H11 complete
                        nc.vector.tensor_tensor(h11, h11, p_, AL.add)
                        nc.scalar.activation(hT[:, j, 0:TS], h11, act_fn)
                    elif mi == 2:  # M3 done -> H12, H22 complete
                        nc.vector.tensor_tensor(h12, h12, p_, AL.add)
                        nc.scalar.activation(hT[:, j, TS:2 * TS], h12, act_fn)
                        if bot:
                            nc.vector.tensor_tensor(h22, h22, p_, AL.add)
                            nc.vector.tensor_tensor(h22, h22, m2sb[:, j, :],
                                                    AL.subtract)
                            nc.scalar.activation(hT[:, JH + j, TS:2 * TS],
                                                 h22, act_fn)
                if bot:
                    # this group produced hT tiles j and 9+j: build the
                    # kk=j pieces of all five layer-2 h-combos now
                    nc.vector.tensor_tensor(hc1[:, j, :], hT[:, j, 0:TS],
                                            hT[:, JH + j, TS:2 * TS], AL.add)
                    nc.vector.tensor_tensor(hc3[:, j, :], hT[:, j, TS:2 * TS],
                                            hT[:, JH + j, TS:2 * TS],
                                            AL.subtract)
                    nc.vector.tensor_tensor(hc4[:, j, :], hT[:, JH + j, 0:TS],
                                            hT[:, j, 0:TS], AL.subtract)
                    nc.vector.tensor_tensor(hc6[:, j, :], hT[:, j, 0:TS],
                                            hT[:, j, TS:2 * TS], AL.add)
                    nc.vector.tensor_tensor(hc7[:, j, :], hT[:, JH + j, 0:TS],
                                            hT[:, JH + j, TS:2 * TS], AL.add)
                else:
                    nc.vector.tensor_tensor(hc6[:, 8, :], hT[:, 8, 0:TS],
                                            hT[:, 8, TS:2 * TS], AL.add)

            # Layer 2: Strassen over [2048, 2304] @ [2304, 1024].
            # Quadrants: rows 1024|1024 (8 n-tiles each), K 1152|1152
            # (9 hT tiles each; bottom tile 17 is all padding and never
            # materialized: its combo pieces collapse to raw hT[:,8,:]
            # slices, a host-negated k-block (N4), or skipped k-tiles
            # (N5, N7)). Out quadrants fold in SBUF f32 and DMA directly.
            L2_ORDER = [1, 5, 3, 4, 2, 0, 6]  # N2 N6 N4 N5 N3 N1 N7
            rhs2 = {
                1: lambda kk: hT[:, kk, 0:TS],            # HH11 raw
                5: lambda kk: hc6[:, kk, :],
                3: lambda kk: hc4[:, kk, :] if kk < 8 else hT[:, 8, 0:TS],
                4: lambda kk: hT[:, JH + kk, TS:2 * TS],  # HH22 raw, 8 kk
                0: lambda kk: hc1[:, kk, :] if kk < 8 else hT[:, 8, 0:TS],
                6: lambda kk: hc7[:, kk, :],              # 8 kk
                2: lambda kk: hc3[:, kk, :] if kk < 8 else hT[:, 8, TS:2 * TS],
            }
            nk2 = {1: 9, 5: 9, 3: 9, 4: 8, 0: 9, 6: 8, 2: 9}
            for n in range(NT // 2):
                w2t = {}
                for mi in L2_ORDER:
                    w2t[mi] = w2p.tile([P, JH, P], f16, tag="w2",
                                       name=f"w2s{mi}_{n}")
                    nc.scalar.dma_start(w2t[mi], w2s[mi, n])
                for mi in L2_ORDER:
                    ps = pp.tile([P, TS], f32, tag="ps", name=f"nps{mi}_{n}")
                    for kk in range(nk2[mi]):
                        nc.tensor.matmul(ps, lhsT=w2t[mi][:, kk, :],
                                         rhs=rhs2[mi](kk),
                                         start=(kk == 0),
                                         stop=(kk == nk2[mi] - 1))
                    if mi == 1:    # N2
                        o21 = hap.tile([P, TS], f32, tag="ha", name=f"o21_{n}")
                        nc.vector.tensor_copy(o21, ps)
                    elif mi == 5:  # N6
                        o22 = hap.tile([P, TS], f32, tag="ha", name=f"o22_{n}")
                        nc.vector.tensor_copy(o22, ps)
                        nc.vector.tensor_tensor(o22, o22, o21, AL.subtract)
                    elif mi == 3:  # N4 -> O21 done
                        o11 = hap.tile([P, TS], f32, tag="ha", name=f"o11_{n}")
                        nc.vector.tensor_tensor(o21, o21, ps, AL.add)
                        nc.sync.dma_start(outt[NT // 2 + n, :, 0:TS], o21)
                        nc.vector.tensor_copy(o11, ps)
                    elif mi == 4:  # N5
                        o12 = hap.tile([P, TS], f32, tag="ha", name=f"o12_{n}")
                        nc.vector.tensor_tensor(o11, o11, ps, AL.subtract)
                        nc.vector.tensor_copy(o12, ps)
                    elif mi == 2:  # N3 -> O12 done
                        nc.vector.tensor_tensor(o12, o12, ps, AL.add)
                        nc.sync.dma_start(outt[n, :, TS:2 * TS], o12)
                        nc.vector.tensor_tensor(o22, o22, ps, AL.add)
                    elif mi == 0:  # N1 -> O22 done
                        nc.vector.tensor_tensor(o11, o11, ps, AL.add)
                        nc.vector.tensor_tensor(o22, o22, ps, AL.add)
                        nc.sync.dma_start(outt[NT // 2 + n, :, TS:2 * TS], o22)
                    elif mi == 6:  # N7 -> O11 done, the only post-last-matmul
                        nc.vector.tensor_tensor(o11, o11, ps, AL.add)
                        nc.sync.dma_start(outt[n, :, 0:TS], o11)

    nc.compile()
    return nc


def _get_nc_strassen(Tc):
    key = ("strassen", Tc)
    if key not in _NC_CACHE:
        _NC_CACHE[key] = _build_nc_strassen(Tc)
    return _NC_CACHE[key]


def _pack_w1(W1, Hg, npdt):
    # W1 [Hk, D] -> padded [Hg, D] -> w1d[j,p,k,c] = W1[j*P+c, k*P+p]
    Hk, D = W1.shape
    W1p = np.zeros((Hg, D), np.float32)
    W1p[:Hk] = W1
    # [JT, P(c), KD, P(p)] <- transpose of [JT,c,KD,p] from reshape
    a = W1p.reshape(Hg // P, P, D // P, P).transpose(0, 3, 2, 1)
    return np.ascontiguousarray(a).astype(npdt)


def _pack_w2(W2T, Hg, npdt):
    # W2T [Hk, D] (= w2[:, kb].T) -> padded [Hg, D]
    # w2d[n,p,j,c] = W2T[j*P+p, n*P+c]
    Hk, D = W2T.shape
    W2p = np.zeros((Hg, D), np.float32)
    W2p[:Hk] = W2T
    a = W2p.reshape(Hg // P, P, D // P, P).transpose(2, 1, 0, 3)
    return np.ascontiguousarray(a).astype(npdt)


def _pack_x(xc, npdt):
    # xc [Tc, D] -> xt[p,k,t] = xc[t, k*P+p]
    Tc, D = xc.shape
    a = xc.T.reshape(D // P, P, Tc).transpose(1, 0, 2)
    return np.ascontiguousarray(a).astype(npdt)


def kernel(x, w1, w2, mask1, mask2, _trace=False):
    mode = MM_DTYPE
    npdt = _NPDT[mode]

    x = np.asarray(x, np.float32)
    w1 = np.asarray(w1, np.float32)
    w2 = np.asarray(w2, np.float32)
    mask1 = np.asarray(mask1, np.float32)
    mask2 = np.asarray(mask2, np.float32)

    B, S, D = x.shape
    T = B * S
    H = w1.shape[0]
    x2 = x.reshape(T, D)

    # Sharding of the hidden dimension: keep only hidden units whose
    # mask1 row and mask2 column are nonzero (the rest contribute exactly
    # zero). Requires whole-row / whole-column masks, which is what this
    # module's sparsity pattern guarantees; otherwise fall back to dense.
    structured = bool((mask1 == mask1[:, :1]).all()) and bool(
        (mask2 == mask2[:1, :]).all()
    )
    if structured:
        k1 = np.flatnonzero(mask1[:, 0])
        k2 = np.flatnonzero(mask2[0, :])
        kb = np.intersect1d(k1, k2)
        if kb.size == 0:
            return np.zeros((B, S, D), np.float32)
        W1 = w1[kb]             # [Hk, D], mask1 rows are all-ones here
        W2T = w2[:, kb].T       # [Hk, D], mask2 cols are all-ones here
    else:
        W1 = w1 * mask1
        W2T = (w2 * mask2).T
        if mode in ("f32r", "f32"):
            mode = "f16"        # dense fallback: halve SBUF footprint
            npdt = _NPDT[mode]
    Hk = W1.shape[0]
    Hg = max(P, ((Hk + P - 1) // P) * P)

    assert T % N_CORES == 0
    Tc = T // N_CORES

    # Strassen layer-1 path: fixed shapes (D=2048, Tc=1024), f16, hidden
    # fits in 2x1152 with the standard 17-tile layer-2 (Hk <= 2176).
    use_strassen = (
        structured and mode == "f16" and D == 2048 and Tc == 1024
        and Hk <= 2176
        and os.environ.get("BASS_MLP_STRASSEN", "1") == "1"
    )

    global LAST_RESULT
    if use_strassen:
        Hh, Dh = 1152, 1024
        Wp = np.zeros((2 * Hh, D), np.float32)
        Wp[:Hk] = W1
        W11 = Wp[:Hh, :Dh]
        W12 = Wp[:Hh, Dh:]
        W21 = Wp[Hh:, :Dh]
        W22 = Wp[Hh:, Dh:]
        combos = [W11 + W22, W21 + W22, W11, W22, W11 + W12,
                  W21 - W11, W12 - W22]
        w1s = np.stack([_pack_w1(c, Hh, npdt) for c in combos])
        # layer-2 Strassen weight combos over W2g [D, 2304]
        W2g = np.zeros((D, 2 * Hh), np.float32)
        W2g[:, :Hk] = W2T.T
        A11 = W2g[:Dh, :Hh]
        A12 = W2g[:Dh, Hh:]
        A21 = W2g[Dh:, :Hh]
        A22 = W2g[Dh:, Hh:]
        combos2 = [A11 + A22, A21 + A22, A11, A22.copy(), A11 + A12,
                   A21 - A11, A12 - A22]
        # N4 contracts HC4 = HH21-HH11; its kk=8 piece is -HH11[8] (tile 17
        # is padding), realized by negating A22's kk=8 column block and
        # feeding raw hT[:,8,0:TS]
        combos2[3][:, 8 * P:] *= -1.0
        w2s = np.stack([_pack_w2(np.ascontiguousarray(c.T), Hh, npdt)
                        for c in combos2])
        nc = _get_nc_strassen(Tc)
        in_maps = []
        for c in range(N_CORES):
            t0 = c * Tc
            in_maps.append({
                "xt": _pack_x(x2[t0:t0 + Tc], npdt),
                "w1s": w1s,
                "w2s": w2s,
            })
        res = run_bass_kernel_spmd(
            nc, in_maps, core_ids=list(range(N_CORES)), trace=_trace,
        )
        LAST_RESULT = res
        out = np.empty((T, D), np.float32)
        for c in range(N_CORES):
            t0 = c * Tc
            o = res.results[c]["outt"]  # [NT, P, Tc]
            out[t0:t0 + Tc] = o.reshape(D, Tc).T
        return out.reshape(B, S, D)

    w1d = _pack_w1(W1, Hg, npdt)
    w2d = _pack_w2(W2T, Hg, npdt)

    # Token-parallel over cores, in sequential rounds if a full token
    # slice per core would not divide into TS chunks or not fit in SBUF
    # (x + hT are SBUF-resident: (KD + JT) * Tc * itemsize per partition).
    itemsz = np.dtype(npdt).itemsize
    rounds = 1
    while (Tc // rounds) % TS != 0 or (Tc // rounds) == 0 or (
        (D // P + Hg // P) * (Tc // rounds) * itemsz > 140 * 1024
    ):
        rounds *= 2
        assert rounds <= 16, "input too large for SBUF tiling scheme"
    Tc //= rounds

    nc = _get_nc(D, Hg, Tc, mode)

    out = np.empty((T, D), np.float32)
    for r in range(rounds):
        in_maps = []
        for c in range(N_CORES):
            t0 = (r * N_CORES + c) * Tc
            in_maps.append({
                "xt": _pack_x(x2[t0:t0 + Tc], npdt),
                "w1d": w1d,
                "w2d": w2d,
            })
        res = run_bass_kernel_spmd(
            nc, in_maps, core_ids=list(range(N_CORES)), trace=_trace,
        )
        LAST_RESULT = res
        for c in range(N_CORES):
            t0 = (r * N_CORES + c) * Tc
            o = res.results[c]["outt"]  # [NT, P, Tc]
            out[t0:t0 + Tc] = o.reshape(D, Tc).T

    return out.reshape(B, S, D)

